# revision 78
# baseline (speedup 1.0000x reference)
"""Trainium2 Bass kernel for nn_DistLoss_18949395710456 (retrieval_knn).

Computation (see reference): for each (b, l) stroke pair, gather a "pooled"
color from the ref image at the predicted position, build the L1 color
similarity map over all 256x256 pixels, take the top-8 closest pixels,
convert winners to normalized coords, distance from stroke l+1's predicted
position to stroke l's candidates, min over the 8 candidates, mean over
(b, l=1..127) -> scalar.

v2 architecture (all fp32, exact selection):
  - 4 pairs packed per (128, 2048) tile: partition p serves pair p//32,
    pixel_flat = (p%32)*2048 + f.  Amortizes the ~280ns fixed overhead of
    every ScalarE activation over 4x more work.
  - group-max pruning: the field -(|r0-c0|+|r1-c1|+|r2-c2|) is max-pooled
    over groups of 16 contiguous pixels.  The exact top-8 pixels provably
    live in the top-8 groups (by group max), so MAX8/FIND_INDEX8 run on the
    8x smaller gmax tile instead of the full field.
  - winner windows (8 groups x 16 px) are re-gathered from DRAM (indirect
    DMA on a host-precomputed (4096, 48) group-major layout) and the exact
    f32 similarity is recomputed for those <=128 candidates per pair.
  - final: per-pair top-8 of the 128 candidates by value threshold (8th
    largest), distances masked by the threshold, min, sqrt.

Engine split per pack: ScalarE 3 abs-activations; GpSimd s01 = -d0-d1;
DVE s2 = s01-d2, pool_max, MAX8, FIND_INDEX8, claims; PE selector matmul
and claim-resolution transposes.

Sharding: identical to baseline: 2 cores per image b (core 2b: l=0..63;
core 2b+1: l=64..126 plus a padded duplicate), host averages.
"""

import sys

sys.path.insert(0, "/opt/trn_rl_repo")

import numpy as np

import concourse.bass as bass
import concourse.bacc as bacc
import concourse.mybir as mybir
from concourse.bass import IndirectOffsetOnAxis
from concourse.masks import make_identity
from concourse.tile import TileContext

F32 = mybir.dt.float32
F16 = mybir.dt.float16
U16 = mybir.dt.uint16
U32 = mybir.dt.uint32
ALU = mybir.AluOpType
ACTF = mybir.ActivationFunctionType
AX = mybir.AxisListType

P = 128
FD = 2048          # free dim of a pack tile
K4 = 4             # pairs per pack
NPACK = 16
NPAIR = 64         # pairs per core
G = 16             # pixels per group
NGROW = FD // G    # groups per partition row = 128
NGPAIR = 32 * NGROW  # groups per pair = 4096
IMG = 256
MAGIC = 12582912.0           # 1.5 * 2^23: rne to integer
FLOOR16 = -0.46875           # rne(v + this) == floor(v) for v = k + m/16
BIG = 1.0e9

N_CORES = 8

_cached = {}


def _build_program():
    nc = bacc.Bacc(
        "TRN2",
        target_bir_lowering=False,
        debug=False,
        enable_asserts=False,
        num_devices=N_CORES,
    )
    # image as (ch, 32, 2048): pixel_flat = p*2048 + f
    imgq = nc.dram_tensor("imgq", [3, 32, FD], F16, kind="ExternalInput").ap()
    # group-major window layout: imgwin[g] = [ch0 16px, ch1 16px, ch2 16px]
    imgwin = nc.dram_tensor("imgwin", [NGPAIR, 3 * G], F32, kind="ExternalInput").ap()
    # pooled-color bias tables, host-gathered (pure indexing of the inputs):
    # cbp[ch][32j+p, m] = c_ch(pair 4m+j); cwt[ch][32k+8j+rk, c] = c_ch(16c+4k+j)
    cbpd = [nc.dram_tensor(f"cbp{ch}", [P, NPACK], F32, kind="ExternalInput").ap()
            for ch in range(3)]
    cwtd = [nc.dram_tensor(f"cwt{ch}", [P, 4], F32, kind="ExternalInput").ap()
            for ch in range(3)]
    npx8 = nc.dram_tensor("npx8", [NPAIR, 8], F32, kind="ExternalInput").ap()
    npy8 = nc.dram_tensor("npy8", [NPAIR, 8], F32, kind="ExternalInput").ap()
    goffd = nc.dram_tensor("goff", [P, 1], F32, kind="ExternalInput").ap()
    px256d = nc.dram_tensor("px256", [16, G], F32, kind="ExternalInput").ap()
    b0td = nc.dram_tensor("b0t", [K4, P], F32, kind="ExternalInput").ap()
    epsrd = nc.dram_tensor("epsr", [P, NGROW], F32, kind="ExternalInput").ap()
    out = nc.dram_tensor("out", [NPAIR], F32, kind="ExternalOutput").ap()

    from contextlib import ExitStack

    with TileContext(nc) as tc, ExitStack() as ctx:
        consts = ctx.enter_context(tc.tile_pool(name="consts", bufs=1))
        dpool = ctx.enter_context(tc.tile_pool(name="dpool", bufs=3))
        spool = ctx.enter_context(tc.tile_pool(name="spool", bufs=2))
        gpool = ctx.enter_context(tc.tile_pool(name="gpool", bufs=3))
        small = ctx.enter_context(tc.tile_pool(name="small", bufs=4))
        wpool = ctx.enter_context(tc.tile_pool(name="wpool", bufs=2))
        psum = ctx.enter_context(tc.tile_pool(name="psum", bufs=2, space="PSUM"))
        psum1 = ctx.enter_context(tc.tile_pool(name="psum1", bufs=2, space="PSUM"))

        # ---- one-time setup ----

        goff = consts.tile([P, 1], F32)
        nc.sync.dma_start(out=goff[:], in_=goffd)
        px16 = consts.tile([16, G], F32)
        nc.scalar.dma_start(out=px16[:], in_=px256d)
        b0t = consts.tile([K4, P], F32)
        nc.gpsimd.dma_start(out=b0t[:], in_=b0td)
        epsr = consts.tile([P, NGROW], F32)
        nc.sync.dma_start(out=epsr[:], in_=epsrd)
        npxc = []
        npyc = []
        for c in range(4):
            nx = consts.tile([16, 8], F32, tag=f"npxc{c}")
            nc.sync.dma_start(out=nx[:], in_=npx8[16 * c : 16 * c + 16, :])
            npxc.append(nx)
            ny = consts.tile([16, 8], F32, tag=f"npyc{c}")
            nc.scalar.dma_start(out=ny[:], in_=npy8[16 * c : 16 * c + 16, :])
            npyc.append(ny)

        cbp = []
        cwt = []
        for ch in range(3):
            cb = consts.tile([P, NPACK], F32, tag=f"cbp{ch}")
            nc.gpsimd.dma_start(out=cb[:], in_=cbpd[ch])
            cbp.append(cb)
            cww = consts.tile([P, 4], F32, tag=f"cwt{ch}")
            nc.gpsimd.dma_start(out=cww[:], in_=cwtd[ch])
            cwt.append(cww)

        # image planes, x4 replicated across partition quarters.  Many small
        # DMAs (one per quarter x column-half, channel-sequential) so several
        # DMA engines stream in parallel and r0 lands within ~8us.
        r = []
        queues = [nc.sync, nc.scalar, nc.gpsimd]
        for c in range(3):
            rc = consts.tile([P, FD], F16, tag=f"r{c}")
            r.append(rc)
        qi = 0
        for c in range(3):
            hw = imgq[c].rearrange("p (h f) -> p h f", h=2)
            for q in range(4):
                for h in range(2):
                    queues[qi % 3].dma_start(
                        out=r[c][32 * q : 32 * q + 32,
                                 FD // 2 * h : FD // 2 * (h + 1)],
                        in_=hw[:, h])
                    qi += 1

        ident = consts.tile([P, P], F32)
        make_identity(nc, ident)



        # ---- per-pack main pipeline ----
        def pack_big(m):
            d0 = dpool.tile([P, FD], F16, tag="d0")
            d1 = dpool.tile([P, FD], F16, tag="d1")
            d2 = dpool.tile([P, FD], F16, tag="d2")
            nc.scalar.activation(d0[:], r[0][:], ACTF.Abs,
                                 bias=cbp[0][:, m : m + 1], scale=-1.0)
            nc.scalar.activation(d1[:], r[1][:], ACTF.Abs,
                                 bias=cbp[1][:, m : m + 1], scale=-1.0)
            nc.scalar.activation(d2[:], r[2][:], ACTF.Abs,
                                 bias=cbp[2][:, m : m + 1], scale=-1.0)
            s01 = spool.tile([P, FD], F16, tag="s01")
            nc.gpsimd.tensor_tensor(out=s01[:], in0=d0[:], in1=d1[:], op=ALU.add)
            s2 = spool.tile([P, FD], F16, tag="s2")
            nc.gpsimd.tensor_tensor(out=s2[:, :512], in0=s01[:, :512],
                                    in1=d2[:, :512], op=ALU.add)
            nc.vector.tensor_tensor(out=s2[:, 512:], in0=s01[:, 512:],
                                    in1=d2[:, 512:], op=ALU.add)
            gmaxr = gpool.tile([P, NGROW], F32, tag="gmaxr")
            nc.vector.tensor_reduce(
                out=gmaxr[:], in_=s2[:].rearrange("p (g w) -> p g w", w=G),
                axis=AX.X, op=ALU.min, negate=True,
            )
            # subtract a tiny per-group ramp so every gmax value is unique:
            # the claim path then always picks 8 distinct groups
            gmax = gpool.tile([P, NGROW], F32, tag="gmax")
            nc.vector.tensor_tensor(out=gmax[:], in0=gmaxr[:], in1=epsr[:],
                                    op=ALU.subtract)
            cand8 = gpool.tile([P, 8], F32, tag="cand8")
            nc.vector.max(out=cand8[:], in_=gmax[:])
            # merge the pack's per-partition candidates: row j = pair's 256
            gf4 = small.tile([K4, 256], F32, tag="gf4")
            nc.sync.dma_start(out=gf4[:], in_=cand8[:])
            return gmax, gf4

        def pack_small(m, gmax, gf4, woffpf, wbc):
            gwin = small.tile([K4, 8], F32, tag="gwin")
            nc.vector.max(out=gwin[:], in_=gf4[:])
            pgwb = psum.tile([P, 8], F32, tag="pgwb")
            nc.tensor.matmul(pgwb[:], b0t[:], gwin[:])
            midx = gpool.tile([P, 8], U16, tag="midx")
            nc.vector.max_index(out=midx[:], in_max=pgwb[:], in_values=gmax[:])
            clms = small.tile([P, 8], F32, tag="clms")
            nc.scalar.activation(clms[:], midx[:], ACTF.Identity,
                                 bias=goff[:, 0:1])
            # winner group ids: min claim across each pair's 32 partitions
            fT8 = psum1.tile([8, P], F32, tag="fT8")
            nc.tensor.transpose(fT8[:], clms[:], ident[:])
            gpos = small.tile([8, K4], F32, tag="gpos")
            nc.vector.tensor_reduce(
                out=gpos[:], in_=fT8[:].rearrange("q (j p) -> q j p", p=32),
                axis=AX.X, op=ALU.min,
            )
            # pair-major (4,8) group ids for the tail
            gposT = psum.tile([K4, 8], F32, tag="gposT")
            nc.tensor.transpose(gposT[:], gpos[:], ident[0:8, 0:8])
            gposS = small.tile([K4, 8], F32, tag="gposS")
            nc.scalar.copy(gposS[:], gposT[:])
            k = m % 4
            nc.scalar.dma_start(out=wbc[4 * k : 4 * k + 4, :], in_=gposS[:])
            # chunk-instance offsets at partitions 32k + 8j + rank (flat order)
            nc.sync.dma_start(out=woffpf[32 * k : 32 * k + 32, :], in_=gposS[:])

        # ---- per-chunk (16 pairs = 4 packs) window gather + re-rank ----
        def chunk_gather(c, woffpf):
            woffp = small.tile([P, 1], U32, tag="woffp")
            nc.vector.tensor_scalar(woffp[:], woffpf[:], 1.0, 4095.0,
                                    op0=ALU.mult, op1=ALU.min)
            # gather winner windows (3ch x 16px per instance) from DRAM
            wr = wpool.tile([P, 3 * G], F32, tag="wr")
            nc.gpsimd.indirect_dma_start(
                out=wr[:],
                out_offset=None,
                in_=imgwin,
                in_offset=IndirectOffsetOnAxis(ap=woffp[:, :1], axis=0),
            )
            return wr

        def chunk_compute(c, wr, kwmc):
            # per-instance colors come host-prearranged in cwt[ch][:, c]
            aw = []
            for ch in range(3):
                a = wpool.tile([P, G], F32, tag=f"aw{ch}")
                nc.scalar.activation(a[:], wr[:, G * ch : G * ch + G], ACTF.Abs,
                                     bias=cwt[ch][:, c : c + 1], scale=-1.0)
                aw.append(a)
            s01w = wpool.tile([P, G], F32, tag="s01w")
            nc.gpsimd.tensor_tensor(out=s01w[:], in0=aw[0][:], in1=aw[1][:],
                                    op=ALU.add)
            kwwp = wpool.tile([P, G], F32, tag="kwwp")
            nc.gpsimd.tensor_tensor(out=kwwp[:], in0=s01w[:], in1=aw[2][:],
                                    op=ALU.add)
            kww = wpool.tile([P, G], F32, tag="kww")
            nc.vector.tensor_scalar_mul(kww[:], kwwp[:], -1.0)
            # merge: kwmc rows = the chunk's 16 pairs (flat (j, rank, x) order)
            for k in range(4):
                (nc.sync if k % 2 == 0 else nc.scalar).dma_start(
                    out=kwmc[4 * k : 4 * k + 4, :],
                    in_=kww[32 * k : 32 * k + 32, :],
                )

        def chunk_tail(c, kwmc, wbc):
            # exact distances over threshold-selected candidates (16 pairs)
            w8 = small.tile([16, 8], F32, tag="w8")
            nc.vector.max(out=w8[:], in_=kwmc[:])
            t16 = small.tile([16, 8], F32, tag="t16")
            nc.vector.tensor_scalar_mul(t16[:], wbc[:], 0.0625)
            t16b = small.tile([16, 8], F32, tag="t16b")
            nc.vector.tensor_scalar(t16b[:], t16[:], FLOOR16, MAGIC,
                                    op0=ALU.add, op1=ALU.add)
            yy = small.tile([16, 8], F32, tag="yy")
            nc.vector.tensor_scalar_sub(yy[:], t16b[:], MAGIC)  # yy = g // 16
            xfrac = small.tile([16, 8], F32, tag="xfrac")
            nc.gpsimd.tensor_tensor(out=xfrac[:], in0=t16[:], in1=yy[:],
                                    op=ALU.subtract)
            ynorm = small.tile([16, 8], F32, tag="ynorm")
            nc.vector.tensor_scalar_mul(ynorm[:], yy[:], 0.00390625)
            dxb = small.tile([16, 8], F32, tag="dxb")
            nc.gpsimd.tensor_tensor(out=dxb[:], in0=npxc[c][:], in1=xfrac[:],
                                    op=ALU.subtract)
            dyb = small.tile([16, 8], F32, tag="dyb")
            nc.gpsimd.tensor_tensor(out=dyb[:], in0=npyc[c][:], in1=ynorm[:],
                                    op=ALU.subtract)
            dyb2 = small.tile([16, 8], F32, tag="dyb2")
            nc.gpsimd.tensor_tensor(out=dyb2[:], in0=dyb[:], in1=dyb[:],
                                    op=ALU.mult)
            dx = small.tile([16, 128], F32, tag="dx")
            nc.vector.tensor_tensor(
                out=dx[:].rearrange("p (rk x) -> p rk x", rk=8),
                in0=dxb[:].unsqueeze(2).broadcast_to([16, 8, G]),
                in1=px16[:].unsqueeze(1).broadcast_to([16, 8, G]),
                op=ALU.subtract,
            )
            dx2 = small.tile([16, 128], F32, tag="dx2")
            nc.vector.tensor_tensor(out=dx2[:], in0=dx[:], in1=dx[:], op=ALU.mult)
            d2t = small.tile([16, 128], F32, tag="d2t")
            nc.vector.tensor_tensor(
                out=d2t[:].rearrange("p (rk x) -> p rk x", rk=8),
                in0=dx2[:].rearrange("p (rk x) -> p rk x", rk=8),
                in1=dyb2[:].unsqueeze(2).broadcast_to([16, 8, G]),
                op=ALU.add,
            )
            maskI = small.tile([16, 128], F32, tag="maskI")
            nc.vector.tensor_scalar(maskI[:], kwmc[:], w8[:, 7:8], 0.0,
                                    op0=ALU.is_lt, op1=ALU.add)
            e = small.tile([16, 128], F32, tag="e")
            nc.vector.scalar_tensor_tensor(
                out=e[:], in0=maskI[:], scalar=-BIG, in1=d2t[:],
                op0=ALU.mult, op1=ALU.subtract,
            )
            md2c = small.tile([16, 1], F32, tag="md2c")
            nc.vector.tensor_reduce(out=md2c[:], in_=e[:], axis=AX.X,
                                    op=ALU.max, negate=True)
            nc.sync.dma_start(out=md2all[16 * c : 16 * c + 16, :], in_=md2c[:])

        # staged software pipeline: emit pack m's big field ops, then pack
        # m-1's cheap resolution, then (delayed 2+ packs so the gather and
        # window data are ready when the in-order engine queues reach them)
        # the window gather / recompute / distance tail for finished chunks.
        woffs = []
        kwmcs = []
        wbcs = []
        for c in range(4):
            wof = gpool.tile([P, 1], F32, tag=f"woffpf{c}")
            woffs.append(wof)
            kc = gpool.tile([16, 128], F32, tag=f"kwmc{c}")
            kwmcs.append(kc)
            wc = gpool.tile([16, 8], F32, tag=f"wbc{c}")
            wbcs.append(wc)
        md2all = consts.tile([NPAIR, 1], F32)

        pending = None
        wrs = {}
        for m in range(NPACK + 7):
            if m < NPACK:
                big = pack_big(m)
            if pending is not None and m >= 1:
                pm = m - 1
                if pm < NPACK:
                    pack_small(pm, *pending, woffs[pm // 4], wbcs[pm // 4])
            if m >= 5 and (m - 5) % 4 == 0 and (m - 5) // 4 < 4:
                g = (m - 5) // 4
                wrs[g] = chunk_gather(g, woffs[g])
            if m >= 7 and (m - 7) % 4 == 0 and (m - 7) // 4 < 4:
                g = (m - 7) // 4
                chunk_compute(g, wrs.pop(g), kwmcs[g])
            if m >= 9 and (m - 9) % 4 == 0 and (m - 9) // 4 < 4:
                g = (m - 9) // 4
                chunk_tail(g, kwmcs[g], wbcs[g])
            if m < NPACK:
                pending = big

        # final: one sqrt over all pairs, one output DMA
        val = consts.tile([NPAIR, 1], F32)
        nc.scalar.activation(val[:], md2all[:], ACTF.Sqrt)
        nc.sync.dma_start(out=out.rearrange("(p o) -> p o", o=1), in_=val[:])

    nc.compile()
    return nc


def _get_program():
    if "nc" not in _cached:
        _cached["nc"] = _build_program()
    return _cached["nc"]


def make_in_maps(predictions: np.ndarray, ref_imgs: np.ndarray):
    """Shard full inputs into 8 per-core input dicts (pure reindexing)."""
    bs, L, _ = predictions.shape
    pp = predictions[:, :, :2]
    grid = np.ascontiguousarray(pp.reshape(bs * L, 2))
    # pooled-color pixel indices, exactly the reference's grid_sample math
    gix = np.clip(np.round(grid[:, 0] * IMG - 0.5), 0, IMG - 1).astype(np.int64)
    giy = np.clip(np.round(grid[:, 1] * IMG - 0.5), 0, IMG - 1).astype(np.int64)
    gq = giy * IMG + gix  # flat pixel per grid row
    goff = ((np.arange(P, dtype=np.float32) % 32) * NGROW).reshape(P, 1)
    px256 = (np.arange(G, dtype=np.float32) / IMG)[None, :].repeat(16, 0)
    b0t = np.zeros((K4, P), dtype=np.float32)
    for k in range(K4):
        b0t[k, 32 * k : 32 * k + 32] = 1.0
    gid = ((np.arange(P) % 32)[:, None] * NGROW
           + np.arange(NGROW)[None, :]).astype(np.float32)
    epsr = (gid * (2.0 ** -20)).astype(np.float32)
    in_maps = []
    for core in range(N_CORES):
        b = core // 2
        if core % 2 == 0:
            ls = list(range(0, 64))
        else:
            ls = list(range(64, 127)) + [126]  # 63 real pairs + 1 pad
        fi = [l * bs + b for l in ls]
        nxt = pp[b, [l + 1 for l in ls]]  # (64, 2), pair order
        img = np.ascontiguousarray(ref_imgs[b].reshape(3, 65536).astype(np.float32))
        img16 = img.astype(np.float16)
        imgwin = np.ascontiguousarray(
            img.reshape(3, NGPAIR, G).transpose(1, 0, 2).reshape(NGPAIR, 3 * G))
        cols = img[:, gq[fi]]       # exact colors for the window re-rank
        cols16 = img16[:, gq[fi]].astype(np.float32)  # fp16 colors for the field
        d = {
            "imgq": img16.reshape(3, 32, FD),
            "imgwin": imgwin,
            "npx8": np.ascontiguousarray(
                np.repeat(nxt[:, 0:1], 8, axis=1).astype(np.float32)),
            "npy8": np.ascontiguousarray(
                np.repeat(nxt[:, 1:2], 8, axis=1).astype(np.float32)),
            "goff": goff,
            "px256": np.ascontiguousarray(px256.astype(np.float32)),
            "b0t": b0t,
            "epsr": epsr,
        }
        for ch in range(3):
            # cbp[32j+p, m] = c(pair 4m+j)
            cb = np.empty((P, NPACK), dtype=np.float32)
            for j in range(4):
                cb[32 * j : 32 * j + 32, :] = cols16[ch, (np.arange(NPACK) * 4 + j)][None, :]
            d[f"cbp{ch}"] = cb
            # cwt[32k+8j+rk, c] = c(pair 16c+4k+j)
            cw = np.empty((P, 4), dtype=np.float32)
            for c in range(4):
                for k in range(4):
                    for j in range(4):
                        for rk in range(8):
                            cw[32 * k + 8 * j + rk, c] = cols[ch, 16 * c + 4 * k + j]
            d[f"cwt{ch}"] = cw
        in_maps.append(d)
    return in_maps


def kernel(predictions: np.ndarray, ref_imgs: np.ndarray) -> np.ndarray:
    from concourse.bass_utils import run_bass_kernel_spmd

    predictions = np.asarray(predictions, dtype=np.float32)
    ref_imgs = np.asarray(ref_imgs, dtype=np.float32)
    nc = _get_program()
    in_maps = make_in_maps(predictions, ref_imgs)
    res = run_bass_kernel_spmd(nc, in_maps, core_ids=list(range(N_CORES)))
    rows = []
    for b in range(4):
        rows.append(np.concatenate([
            res.results[2 * b]["out"][:64],
            res.results[2 * b + 1]["out"][:63],
        ]))
    val_down = np.stack(rows)  # (4, 127)
    return np.float32(np.mean(val_down))
